# revision 1
# baseline (speedup 1.0000x reference)
"""AttentionTSSA Trainium2 kernel.

Problem: B=8, N=4096, DIM=1024, H=16, D=64.
  w = (x @ Wqkv.T) viewed as (b, h, n, d)
  w_normed = w / max(||w||_n, 1e-12)           (normalize over sequence axis)
  logits[b,h,n] = temp[h] * sum_d w_normed^2
  Pi = softmax over h
  Pi_norm = Pi / (sum_n Pi + 1e-8)
  dots[b,h,d] = sum_n Pi_norm * w^2
  out = -(w * Pi) * (1 / (1 + dots))
  y = out @ Wout.T + bout

Sharding: data-parallel over batch, one batch element per NeuronCore (8 cores).

Per-core layout: w stored transposed in SBUF as w.T[c, n] with c = h*64 + d on
partitions (8 c-tiles of 128) and n on the free axis (4096). All sequence
reductions become free-axis reductions; the softmax over heads is done in a
small [16, n] layout via PE mask-matmuls; per-head broadcasts back to the
[c, n] layout are PE mask-matmuls with a [16, 128] selection matrix.

Matmuls run in float32r (1 cycle/row at free-dim >= 256, ~1e-4 relative
precision at K=1024 measured on hardware).
"""
import sys

sys.path.insert(0, "/opt/trn_rl_repo")

import numpy as np

import concourse.bacc as bacc
import concourse.bass as bass
import concourse.mybir as mybir
import concourse.tile as tile
from concourse.alu_op_type import AluOpType

F32 = mybir.dt.float32
F32R = mybir.dt.float32r
ACT = mybir.ActivationFunctionType
AX = mybir.AxisListType

import os as _os

B, N, DIM, H, D = 8, 4096, 1024, 16, 64
P16_BUFS = int(_os.environ.get("K_P16_BUFS", "4"))
TMP_BUFS = int(_os.environ.get("K_TMP_BUFS", "3"))
PSA_BUFS = int(_os.environ.get("K_PSA_BUFS", "4"))
PSB_BUFS = int(_os.environ.get("K_PSB_BUFS", "3"))
PSC_BUFS = int(_os.environ.get("K_PSC_BUFS", "1"))
XNAT_BUFS = int(_os.environ.get("K_XNAT_BUFS", "3"))
XT_BUFS = int(_os.environ.get("K_XT_BUFS", "1"))
CT = DIM // 128          # 8 c-tiles (each 2 heads)
KT = DIM // 128          # 8 k-tiles
NCH = N // 512           # 8 n-chunks of 512
NSUB = N // 128          # 32 n-subtiles of 128
EPS_NORM = 1e-12
EPS_PI = 1e-8


def build_nc():
    nc = bacc.Bacc(None)

    x_parts = [
        nc.dram_tensor(f"x{i}", [N // 8, DIM], F32, kind="ExternalInput")
        for i in range(8)
    ]
    wqkvT_p = [
        nc.dram_tensor(f"wqkvT{i}", [DIM // 2, DIM], F32, kind="ExternalInput")
        for i in range(2)
    ]   # (k, c) halves
    woutT_p = [
        nc.dram_tensor(f"woutT{i}", [DIM // 2, DIM], F32, kind="ExternalInput")
        for i in range(2)
    ]   # (c, j) halves
    temp_d = nc.dram_tensor("temp", [H, 1], F32, kind="ExternalInput")
    bout_d = nc.dram_tensor("bout", [1, DIM], F32, kind="ExternalInput")
    ident_d = nc.dram_tensor("ident", [128, 128], F32, kind="ExternalInput")
    maskT_d = nc.dram_tensor("maskT", [128, CT, H], F32, kind="ExternalInput")
    bcastM_d = nc.dram_tensor("bcastM", [H, CT, 128], F32, kind="ExternalInput")
    ones16_d = nc.dram_tensor("ones16", [H, 1], F32, kind="ExternalInput")
    ones1x16_d = nc.dram_tensor("ones1x16", [1, H], F32, kind="ExternalInput")
    parityM_d = nc.dram_tensor("parityM", [H, 128], F32, kind="ExternalInput")
    selH_d = nc.dram_tensor("selH", [H, 8], F32, kind="ExternalInput")
    bcols_d = nc.dram_tensor("bout_cols", [128, 8], F32, kind="ExternalInput")
    y_parts = [
        nc.dram_tensor(f"y{i}", [DIM // 8, N], F32, kind="ExternalOutput")
        for i in range(8)
    ]

    with tile.TileContext(nc) as tc:
        with (
            tc.tile_pool(name="big", bufs=1) as big,          # w, weights, consts
            tc.tile_pool(name="xn", bufs=XNAT_BUFS) as xn,            # x natural tiles
            tc.tile_pool(name="xt", bufs=XT_BUFS) as xtp,           # transposed x chunk
            tc.tile_pool(name="tmp", bufs=TMP_BUFS) as tmp,          # [128,512] transients
            tc.tile_pool(name="p16", bufs=P16_BUFS) as p16,          # [16,512]/[1,512] transients
            tc.tile_pool(name="st", bufs=1) as st,            # small stats
            tc.tile_pool(name="psA", bufs=PSA_BUFS, space="PSUM") as psA,
            tc.tile_pool(name="psB", bufs=PSB_BUFS, space="PSUM") as psB,
            tc.tile_pool(name="psC", bufs=PSC_BUFS, space="PSUM") as psC,
            tc.tile_pool(name="dram", bufs=1, space="DRAM") as dram,
        ):
            # ---------------- constants / weights ----------------
            w_sb = big.tile([128, CT, N], F32R, tag="w")           # 128 KiB/part
            wq_sb = big.tile([128, KT, DIM], F32R, tag="wts")      # 32 KiB/part
            ident = big.tile([128, 128], F32R, tag="ident")
            maskT = big.tile([128, CT, H], F32R, tag="maskT")
            bcastM = big.tile([H, CT, 128], F32R, tag="bcastM")
            ones16 = big.tile([H, 1], F32R, tag="ones16")
            ones1x16 = big.tile([1, H], F32R, tag="ones1x16")
            parityM = big.tile([H, 128], F32, tag="parityM")
            selH = big.tile([H, 8], F32, tag="selH")
            bcols_sb = big.tile([128, 8], F32, tag="bcols")
            temp_sb = big.tile([H, 1], F32, tag="temp")

            nc.sync.dma_start(out=ident, in_=ident_d[:, :].bitcast(F32R))
            nc.sync.dma_start(out=maskT, in_=maskT_d[:, :, :].bitcast(F32R))
            nc.sync.dma_start(out=bcastM, in_=bcastM_d[:, :, :].bitcast(F32R))
            nc.sync.dma_start(out=ones16, in_=ones16_d[:, :].bitcast(F32R))
            nc.sync.dma_start(out=ones1x16, in_=ones1x16_d[:, :].bitcast(F32R))
            nc.sync.dma_start(out=parityM, in_=parityM_d[:, :])
            nc.sync.dma_start(out=selH, in_=selH_d[:, :])
            nc.sync.dma_start(out=bcols_sb, in_=bcols_d[:, :])
            nc.sync.dma_start(out=temp_sb, in_=temp_d[:, :])
            for kt in range(KT):
                wp, wr = divmod(kt * 128, DIM // 2)
                nc.sync.dma_start(
                    out=wq_sb[:, kt],
                    in_=wqkvT_p[wp][wr:wr + 128, :].bitcast(F32R),
                )

            # stats tiles
            norm2_parts = st.tile([128, CT, NCH], F32, tag="n2p")
            dots_parts = st.tile([128, CT, NCH], F32, tag="dtp")
            rsqrt_all = st.tile([128, CT], F32, tag="rsq")
            s_parts = st.tile([H, NCH], F32, tag="sp")
            s_sum = st.tile([H, 1], F32, tag="ss")
            sinv16 = st.tile([H, 1], F32, tag="sinv")

            # DRAM scratch
            pi_dram = dram.tile([H, N], F32, tag="pi")

            # ---------------- stage 1: w.T = Wqkv @ x.T ----------------
            # loop n-chunks of 512; transpose x into [k, n] tiles; 8 c-tiles
            for nn in range(NCH):
                xT = xtp.tile([128, KT, 512], F32R, tag="xT")
                for sub in range(4):
                    ns = nn * 4 + sub          # n-subtile index (128 rows of x)
                    for kh in range(2):
                        x_nat = xn.tile([128, 512], F32R, tag="xnat")
                        xp, xr = divmod(ns * 128, N // 8)
                        nc.sync.dma_start(
                            out=x_nat,
                            in_=x_parts[xp][xr:xr + 128,
                                            kh * 512:(kh + 1) * 512].bitcast(F32R),
                        )
                        for k4 in range(4):
                            kt = kh * 4 + k4
                            tps = psA.tile([128, 128], F32, tag="psA")
                            nc.tensor.transpose(
                                tps.bitcast(F32R),
                                x_nat[:, k4 * 128:(k4 + 1) * 128], ident,
                            )
                            nc.vector.tensor_copy(
                                out=xT[:, kt, sub * 128:(sub + 1) * 128],
                                in_=tps,
                            )
                for ct in range(CT):
                    wps = psB.tile([128, 512], F32, tag="psB")
                    for kt in range(KT):
                        nc.tensor.matmul(
                            wps,
                            wq_sb[:, kt, ct * 128:(ct + 1) * 128],
                            xT[:, kt],
                            start=(kt == 0),
                            stop=(kt == KT - 1),
                        )
                    nc.scalar.copy(
                        out=w_sb[:, ct, nn * 512:(nn + 1) * 512], in_=wps
                    )
                    # norm2 partial: sum_n w^2 over this chunk (ACT Square+accum)
                    ndump = psC.tile([128, 512], F32, tag="psC")
                    nc.scalar.activation(
                        out=ndump,
                        in_=w_sb[:, ct, nn * 512:(nn + 1) * 512].bitcast(F32),
                        func=ACT.Square,
                        accum_out=norm2_parts[:, ct, nn:nn + 1],
                    )

            # rsqrt = 1 / max(sqrt(norm2), 1e-12)
            norm2_c = st.tile([128, CT], F32, tag="n2c")
            nc.vector.tensor_reduce(
                out=norm2_c, in_=norm2_parts, axis=AX.X, op=AluOpType.add
            )
            nc.scalar.activation(out=norm2_c, in_=norm2_c, func=ACT.Sqrt)
            nc.vector.tensor_scalar_max(out=norm2_c, in0=norm2_c,
                                        scalar1=EPS_NORM)
            nc.vector.reciprocal(out=rsqrt_all, in_=norm2_c)

            # ---------------- stage 2: logits, softmax over heads, Pi ----------------
            for nn in range(NCH):
                lps = psA.tile([16, 512], F32, tag="psA")
                for ct in range(CT):
                    u = tmp.tile([128, 512], F32R, tag="tmp")
                    nc.scalar.activation(
                        out=u,
                        in_=w_sb[:, ct, nn * 512:(nn + 1) * 512].bitcast(F32),
                        func=ACT.Square,
                        scale=rsqrt_all[:, ct:ct + 1],
                    )
                    nc.tensor.matmul(
                        lps, maskT[:, ct], u,
                        start=(ct == 0), stop=(ct == CT - 1),
                    )
                # E = exp(temp * logits)  (softmax numerator; logits >= 0, no max-sub)
                pi_c = p16.tile([16, 512], F32R, tag="p16")
                nc.scalar.activation(
                    out=pi_c, in_=lps, func=ACT.Exp, scale=temp_sb[:, 0:1]
                )
                # colsum over heads via ones16 matmul
                csps = psB.tile([1, 512], F32, tag="psB")
                nc.tensor.matmul(csps, ones16, pi_c, start=True, stop=True)
                csinv = p16.tile([1, 512], F32R, tag="p16")
                with nc.allow_low_precision(reason="f32r == f32 bit layout"):
                    nc.vector.reciprocal(out=csinv, in_=csps)
                csb = psC.tile([16, 512], F32, tag="psC")
                nc.tensor.matmul(csb, ones1x16, csinv, start=True, stop=True)
                # Pi = E * csinv ; accumulate S_h partials
                pi2 = p16.tile([16, 512], F32, tag="p16")
                nc.vector.scalar_tensor_tensor(
                    out=pi2,
                    in0=pi_c.bitcast(F32),
                    scalar=1.0,
                    in1=csb,
                    op0=AluOpType.mult,
                    op1=AluOpType.mult,
                )
                sdump = p16.tile([16, 512], F32, tag="p16")
                nc.scalar.activation(
                    out=sdump, in_=pi2, func=ACT.Identity,
                    accum_out=s_parts[:, nn:nn + 1],
                )
                nc.sync.dma_start(out=pi_dram[:, nn * 512:(nn + 1) * 512], in_=pi2)

            # S = sum_n Pi ; sinv = 1/(S + 1e-8)
            nc.vector.tensor_reduce(out=s_sum, in_=s_parts, axis=AX.X,
                                    op=AluOpType.add)
            nc.vector.tensor_scalar_add(out=s_sum, in0=s_sum, scalar1=EPS_PI)
            nc.vector.reciprocal(out=sinv16, in_=s_sum)

            # ---------------- stage 3 pass A: dots ----------------
            for nn in range(NCH):
                pi_a = p16.tile([16, 512], F32, tag="p16")
                nc.sync.dma_start(
                    out=pi_a, in_=pi_dram[:, nn * 512:(nn + 1) * 512]
                )
                spi_c = p16.tile([16, 512], F32R, tag="p16")
                nc.scalar.activation(out=spi_c, in_=pi_a, func=ACT.Sqrt)
                for ct in range(CT):
                    pb = psA.tile([128, 512], F32, tag="psA")
                    nc.tensor.matmul(pb, bcastM[:, ct], spi_c, start=True, stop=True)
                    s_t = tmp.tile([128, 512], F32, tag="tmp")
                    nc.vector.tensor_tensor(
                        out=s_t,
                        in0=w_sb[:, ct, nn * 512:(nn + 1) * 512].bitcast(F32),
                        in1=pb, op=AluOpType.mult,
                    )
                    ddump = psC.tile([128, 512], F32, tag="psC")
                    nc.scalar.activation(
                        out=ddump, in_=s_t, func=ACT.Square,
                        accum_out=dots_parts[:, ct, nn:nn + 1],
                    )

            # negattn[c] = -1 / (1 + dots * sinv)
            negattn = st.tile([128, CT], F32, tag="natn")
            dots_c = st.tile([128, CT], F32, tag="dc")
            sinv_c = st.tile([128, CT], F32, tag="sc")
            nc.vector.tensor_reduce(out=dots_c, in_=dots_parts, axis=AX.X,
                                    op=AluOpType.add)
            sinvSel = st.tile([H, 8], F32, tag="sinvsel")
            nc.vector.tensor_scalar_mul(out=sinvSel, in0=selH, scalar1=sinv16)
            svp = psC.tile([128, 8], F32, tag="psC")
            nc.tensor.matmul(svp, parityM, sinvSel, start=True, stop=True)
            nc.vector.tensor_copy(out=sinv_c, in_=svp)
            nc.vector.tensor_tensor(out=negattn, in0=dots_c, in1=sinv_c,
                                    op=AluOpType.mult)
            nc.vector.tensor_scalar_add(out=negattn, in0=negattn, scalar1=1.0)
            nc.vector.reciprocal(out=negattn, in_=negattn)
            nc.vector.tensor_scalar_mul(out=negattn, in0=negattn, scalar1=-1.0)

            # ---------------- stage 3 pass B: q = -attn * Pi * w (in place) ----------------
            for nn in range(NCH):
                pi_c = p16.tile([16, 512], F32R, tag="p16")
                nc.sync.dma_start(
                    out=pi_c,
                    in_=pi_dram[:, nn * 512:(nn + 1) * 512].bitcast(F32R),
                )
                for ct in range(CT):
                    pb = psA.tile([128, 512], F32, tag="psA")
                    nc.tensor.matmul(pb, bcastM[:, ct], pi_c, start=True, stop=True)
                    nc.vector.scalar_tensor_tensor(
                        out=w_sb[:, ct, nn * 512:(nn + 1) * 512],
                        in0=pb,
                        scalar=negattn[:, ct:ct + 1],
                        in1=w_sb[:, ct, nn * 512:(nn + 1) * 512],
                        op0=AluOpType.mult,
                        op1=AluOpType.mult,
                    )

            # ---------------- stage 4: y.T = Wout @ q.T (+ bout), host untransposes ----------------
            # lhsT = woutT[c, j-subtile] stays stationary across a 4-chunk wave,
            # amortizing weight loads 4x. Bias added in the ACT psum->sbuf copy.
            wout_sb = big.tile([128, CT, DIM], F32R, tag="wts")
            for ct in range(CT):
                wp, wr = divmod(ct * 128, DIM // 2)
                nc.sync.dma_start(
                    out=wout_sb[:, ct],
                    in_=woutT_p[wp][wr:wr + 128, :].bitcast(F32R),
                )
            for jsub in range(8):
                for wave in range(2):
                    yps_list = []
                    for i in range(4):
                        yps_i = psB.tile([128, 512], F32, tag="psB")
                        yps_list.append(yps_i)
                    for ct in range(CT):
                        for i in range(4):
                            nn = wave * 4 + i
                            nc.tensor.matmul(
                                yps_list[i],
                                wout_sb[:, ct, jsub * 128:(jsub + 1) * 128],
                                w_sb[:, ct, nn * 512:(nn + 1) * 512],
                                start=(ct == 0),
                                stop=(ct == CT - 1),
                            )
                    for i in range(4):
                        nn = wave * 4 + i
                        y_sb = tmp.tile([128, 512], F32, tag="tmp")
                        nc.scalar.activation(
                            out=y_sb, in_=yps_list[i], func=ACT.Identity,
                            bias=bcols_sb[:, jsub:jsub + 1],
                        )
                        yp, yr = divmod(jsub * 128, DIM // 8)
                        nc.sync.dma_start(
                            out=y_parts[yp][yr:yr + 128,
                                            nn * 512:(nn + 1) * 512],
                            in_=y_sb,
                        )

    nc.finalize()
    return nc


_NC_CACHE = {}


def _get_nc():
    if "nc" not in _NC_CACHE:
        _NC_CACHE["nc"] = build_nc()
    return _NC_CACHE["nc"]


def make_host_inputs(x, Wqkv, temp, Wout, bout):
    """Build the per-core input maps (host-side sharding + weight transposes)."""
    x = np.ascontiguousarray(np.asarray(x, dtype=np.float32))
    wqkvT = np.ascontiguousarray(np.asarray(Wqkv, dtype=np.float32).T)
    woutT = np.ascontiguousarray(np.asarray(Wout, dtype=np.float32).T)
    temp = np.ascontiguousarray(np.asarray(temp, dtype=np.float32).reshape(H, 1))
    bout2 = np.ascontiguousarray(np.asarray(bout, dtype=np.float32).reshape(1, DIM))
    ident = np.eye(128, dtype=np.float32)
    # maskT[p, ct, h] = 1 iff h == 2*ct + (p >= 64)
    p = np.arange(128)
    maskT = np.zeros((128, CT, H), dtype=np.float32)
    for ct in range(CT):
        maskT[p, ct, 2 * ct + (p >= 64)] = 1.0
    # bcastM[h, ct, p] = maskT[p, ct, h]
    bcastM = np.ascontiguousarray(maskT.transpose(2, 1, 0))
    ones16 = np.ones((H, 1), dtype=np.float32)
    ones1x16 = np.ones((1, H), dtype=np.float32)
    parityM = np.zeros((H, 128), dtype=np.float32)
    for h in range(H):
        parityM[h, :] = ((np.arange(128) >= 64) == (h % 2)).astype(np.float32)
    selH = np.zeros((H, 8), dtype=np.float32)
    for h in range(H):
        selH[h, h // 2] = 1.0
    bout_cols = np.ascontiguousarray(
        np.asarray(bout, dtype=np.float32).reshape(8, 128).T
    )

    shared = {
        "wqkvT0": wqkvT[:DIM // 2], "wqkvT1": wqkvT[DIM // 2:],
        "woutT0": woutT[:DIM // 2], "woutT1": woutT[DIM // 2:],
        "temp": temp, "bout": bout2,
        "ident": ident, "maskT": maskT, "bcastM": bcastM,
        "ones16": ones16, "ones1x16": ones1x16, "bout_cols": bout_cols,
        "parityM": parityM, "selH": selH,
    }
    maps = []
    for b in range(B):
        m = dict(shared)
        for i in range(8):
            m[f"x{i}"] = x[b, i * (N // 8):(i + 1) * (N // 8)]
        maps.append(m)
    return maps


def kernel(x, Wqkv, temp, Wout, bout):
    from concourse.bass_utils import run_bass_kernel_spmd

    nc = _get_nc()
    in_maps = make_host_inputs(x, Wqkv, temp, Wout, bout)
    res = run_bass_kernel_spmd(nc, in_maps, list(range(B)))
    y = np.empty((B, N, DIM), dtype=np.float32)
    for b in range(B):
        yt = np.concatenate(
            [res.results[b][f"y{i}"] for i in range(8)], axis=0
        )
        y[b] = yt.T
    return y



# revision 6
# speedup vs baseline: 1.2094x; 1.2094x over previous
"""AttentionTSSA Trainium2 kernel (v2, bf16 datapath).

Problem: B=8, N=4096, DIM=1024, H=16, D=64.
  w = (x @ Wqkv.T) viewed as (b, h, n, d)
  w_normed = w / max(||w||_n, 1e-12)           (normalize over sequence axis)
  logits[b,h,n] = temp[h] * sum_d w_normed^2
  Pi = softmax over h
  Pi_norm = Pi / (sum_n Pi + 1e-8)
  dots[b,h,d] = sum_n Pi_norm * w^2
  out = -(w * Pi) * (1 / (1 + dots))
  y = out @ Wout.T + bout

Sharding: data-parallel over batch, one batch element per NeuronCore (8 cores).

v2 design vs v1:
  - Host converts x/Wqkv/Wout to bf16. x is transposed into SBUF [k, n]
    layout directly by DMA (InstDmaTransposeAnt, 2-byte dtype), killing the
    PE transposes + DVE psum copies of v1.
  - Stage-1 psum is written twice by Act: Identity -> w_sb (bf16) and
    Square+accum -> w2_sb (bf16) + norm2 partials. w2 feeds the logits
    mask-matmul (rsqrt^2 folded into the mask weights, killing the
    per-chunk normalize pass) and the dots reduction.
  - Pi kept entirely in SBUF ([16, N] bf16); softmax chunk loop also
    computes dots partials (tensor_tensor_reduce) and q' = Pi*w (in place
    of w2) immediately, so stage 3 disappears.
  - negattn = -1/(1+dots/S) is folded into the Wout weights (per-partition
    Act scale); bout is added on the host; stage-4 psum tiles are DMA'd
    straight to DRAM.
"""
import sys

sys.path.insert(0, "/opt/trn_rl_repo")

import os as _os

import numpy as np

import concourse.bacc as bacc
import concourse.bass as bass
import concourse.mybir as mybir
import concourse.tile as tile
from concourse.alu_op_type import AluOpType

F32 = mybir.dt.float32
BF16 = mybir.dt.bfloat16
ACT = mybir.ActivationFunctionType
AX = mybir.AxisListType

B, N, DIM, H, D = 8, 4096, 1024, 16, 64
CT = DIM // 128          # 8 c-tiles (each 2 heads)
KT = DIM // 128          # 8 k-tiles
NCH = N // 512           # 8 n-chunks of 512
EPS_NORM2 = 1e-24        # max(||w||, 1e-12)^2 == max(||w||^2, 1e-24)
EPS_PI = 1e-8

XT_BUFS = int(_os.environ.get("K_XT_BUFS", "2"))
PSA_BUFS = int(_os.environ.get("K_PSA_BUFS", "4"))
PBS_BUFS = int(_os.environ.get("K_PBS_BUFS", "3"))
P16_BUFS = int(_os.environ.get("K_P16_BUFS", "3"))
JUNK_BUFS = int(_os.environ.get("K_JUNK_BUFS", "2"))
# of the 8 (ct) dots reductions per chunk, how many go TT(DVE 2x)+Act-accum
# instead of DVE ttr
DOTS_ACT = int(_os.environ.get("K_DOTS_ACT", "0"))
# of the 8 pb psum->sbuf bf16 copies per chunk, how many go on gpsimd (Pool)
PBS_POOL = int(_os.environ.get("K_PBS_POOL", "0"))


def build_nc():
    nc = bacc.Bacc(None)

    x_parts = [
        nc.dram_tensor(f"x{i}", [512, DIM], BF16, kind="ExternalInput")
        for i in range(NCH)
    ]
    wqkvT_d = nc.dram_tensor("wqkvT", [DIM, DIM], BF16, kind="ExternalInput")
    woutT_d = nc.dram_tensor("woutT", [DIM, DIM], BF16, kind="ExternalInput")
    temp_d = nc.dram_tensor("temp", [H, 1], F32, kind="ExternalInput")
    maskT_d = nc.dram_tensor("maskT", [128, CT, H], F32, kind="ExternalInput")
    bcastM_d = nc.dram_tensor("bcastM", [H, CT, 128], BF16, kind="ExternalInput")
    ones16_d = nc.dram_tensor("ones16", [H, 1], BF16, kind="ExternalInput")
    ones1x16_d = nc.dram_tensor("ones1x16", [1, H], BF16, kind="ExternalInput")
    parityM_d = nc.dram_tensor("parityM", [H, 128], F32, kind="ExternalInput")
    selH_d = nc.dram_tensor("selH", [H, 8], F32, kind="ExternalInput")
    y_parts = [
        nc.dram_tensor(f"y{i}", [128, N], F32, kind="ExternalOutput")
        for i in range(CT)
    ]

    with tile.TileContext(nc) as tc:
        with (
            tc.tile_pool(name="big", bufs=1) as big,
            tc.tile_pool(name="xt", bufs=XT_BUFS) as xtp,
            tc.tile_pool(name="pbs", bufs=PBS_BUFS) as pbsp,
            tc.tile_pool(name="p16", bufs=P16_BUFS) as p16,
            tc.tile_pool(name="junk", bufs=JUNK_BUFS) as junkp,
            tc.tile_pool(name="ysb", bufs=2) as ysbp,
            tc.tile_pool(name="st", bufs=1) as st,
            tc.tile_pool(name="psA", bufs=PSA_BUFS, space="PSUM") as psA,
            tc.tile_pool(name="psS", bufs=1, space="PSUM") as psS,
        ):
            # ---------------- persistent SBUF ----------------
            w_sb = big.tile([128, CT, N], BF16, tag="w")        # 64 KiB/part
            w2_sb = big.tile([128, CT, N], BF16, tag="w2")      # 64 KiB/part
            wq_sb = big.tile([128, KT, DIM], BF16, tag="wq")    # 16 KiB/part
            wout_sb = big.tile([128, CT, DIM], BF16, tag="wout")  # 16 KiB/part
            maskT = big.tile([128, CT, H], F32, tag="maskT")
            maskTs = big.tile([128, CT, H], BF16, tag="maskTs")
            bcastM = big.tile([H, CT, 128], BF16, tag="bcastM")
            ones16 = big.tile([H, 1], BF16, tag="ones16")
            ones1x16 = big.tile([1, H], BF16, tag="ones1x16")
            parityM = big.tile([H, 128], F32, tag="parityM")
            selH = big.tile([H, 8], F32, tag="selH")
            temp_sb = big.tile([H, 1], F32, tag="temp")
            pi_sb = big.tile([H, N], BF16, tag="pi")            # 8 KiB/part

            # stats
            norm2_parts = st.tile([128, CT, NCH], F32, tag="n2p")
            dots_parts = st.tile([128, CT, NCH], F32, tag="dtp")
            rsq2 = st.tile([128, CT], F32, tag="rsq2")
            s_parts = st.tile([H, NCH], F32, tag="sp")
            s_sum = st.tile([H, 1], F32, tag="ss")
            sinv16 = st.tile([H, 1], F32, tag="sinv")

            # ---------------- phase 0: input DMAs ----------------
            # chunk-0 x transpose interleaved with wq so the first matmul can
            # start as early as possible; consts ride the Act dge queue.
            def load_xT(nn, interleave_wq=False):
                t = xtp.tile([128, KT, 512], BF16, tag="xT")
                for kt in range(KT):
                    nc.sync.dma_start_transpose(
                        out=t[:, kt],
                        in_=x_parts[nn][:, kt * 128:(kt + 1) * 128],
                    )
                    if interleave_wq:
                        nc.sync.dma_start(
                            out=wq_sb[:, kt],
                            in_=wqkvT_d[kt * 128:(kt + 1) * 128, :],
                        )
                return t

            xT_next = load_xT(0, interleave_wq=True)

            nc.scalar.dma_start(out=temp_sb, in_=temp_d[:, :])
            nc.scalar.dma_start(out=maskT, in_=maskT_d[:, :, :])
            nc.scalar.dma_start(out=bcastM, in_=bcastM_d[:, :, :])
            nc.scalar.dma_start(out=ones16, in_=ones16_d[:, :])
            nc.scalar.dma_start(out=ones1x16, in_=ones1x16_d[:, :])
            nc.scalar.dma_start(out=parityM, in_=parityM_d[:, :])
            nc.scalar.dma_start(out=selH, in_=selH_d[:, :])
            for ct in range(CT):
                nc.scalar.dma_start(
                    out=wout_sb[:, ct],
                    in_=woutT_d[ct * 128:(ct + 1) * 128, :],
                )

            # ---------------- phase 1: w.T = Wqkv @ x.T ----------------
            for nn in range(NCH):
                xT = xT_next
                if nn + 1 < NCH:
                    xT_next = load_xT(nn + 1)
                ns = slice(nn * 512, (nn + 1) * 512)
                for ct in range(CT):
                    wps = psA.tile([128, 512], F32, tag="psA")
                    for kt in range(KT):
                        nc.tensor.matmul(
                            wps,
                            wq_sb[:, kt, ct * 128:(ct + 1) * 128],
                            xT[:, kt],
                            start=(kt == 0),
                            stop=(kt == KT - 1),
                        )
                    nc.scalar.activation(
                        out=w_sb[:, ct, ns], in_=wps, func=ACT.Identity
                    )
                    nc.scalar.activation(
                        out=w2_sb[:, ct, ns], in_=wps, func=ACT.Square,
                        accum_out=norm2_parts[:, ct, nn:nn + 1],
                    )

            # rsq2 = 1 / max(norm2, 1e-24); maskTs = maskT * rsq2 (bf16)
            norm2_c = st.tile([128, CT], F32, tag="n2c")
            nc.vector.tensor_reduce(
                out=norm2_c, in_=norm2_parts, axis=AX.X, op=AluOpType.add
            )
            nc.vector.tensor_scalar_max(out=norm2_c, in0=norm2_c,
                                        scalar1=EPS_NORM2)
            nc.vector.reciprocal(out=rsq2, in_=norm2_c)
            for ct in range(CT):
                nc.scalar.activation(
                    out=maskTs[:, ct], in_=maskT[:, ct], func=ACT.Identity,
                    scale=rsq2[:, ct:ct + 1],
                )

            # ---------------- phase 2: softmax over heads + dots + q ----------------
            for nn in range(NCH):
                ns = slice(nn * 512, (nn + 1) * 512)
                lps = psS.tile([16, 512], F32, tag="lps")
                for ct in range(CT):
                    nc.tensor.matmul(
                        lps, maskTs[:, ct], w2_sb[:, ct, ns],
                        start=(ct == 0), stop=(ct == CT - 1),
                    )
                # E = exp(temp * logits)  (logits in [0, temp], no max-sub)
                E = p16.tile([16, 512], BF16, tag="E")
                nc.scalar.activation(
                    out=E, in_=lps, func=ACT.Exp, scale=temp_sb[:, 0:1]
                )
                csps = psS.tile([1, 512], F32, tag="csps")
                nc.tensor.matmul(csps, ones16, E, start=True, stop=True)
                csinv = p16.tile([1, 512], BF16, tag="csinv")
                with nc.allow_low_precision(reason="pi normalizer in bf16"):
                    nc.vector.reciprocal(out=csinv, in_=csps)
                csb = psS.tile([16, 512], F32, tag="csb")
                nc.tensor.matmul(csb, ones1x16, csinv, start=True, stop=True)
                # Pi = E * csinv (bf16, kept in SBUF); S partials via Act accum
                nc.vector.tensor_tensor(
                    out=pi_sb[:, ns], in0=E, in1=csb, op=AluOpType.mult,
                )
                sjk = p16.tile([16, 512], BF16, tag="sjunk")
                nc.scalar.activation(
                    out=sjk, in_=pi_sb[:, ns], func=ACT.Identity,
                    accum_out=s_parts[:, nn:nn + 1],
                )
                for ct in range(CT):
                    pb = psA.tile([128, 512], F32, tag="psA")
                    nc.tensor.matmul(pb, bcastM[:, ct], pi_sb[:, ns],
                                     start=True, stop=True)
                    pbs = pbsp.tile([128, 512], BF16, tag="pbs")
                    if ct < PBS_POOL:
                        nc.gpsimd.tensor_copy(out=pbs, in_=pb)
                    else:
                        nc.scalar.activation(out=pbs, in_=pb,
                                             func=ACT.Identity)
                    # dots partial: sum_n w2 * Pi_bc (TT in DVE 2x mode, then
                    # a free-axis reduce on DVE or Act per the split knob)
                    ds = junkp.tile([128, 512], BF16, tag="junk")
                    nc.vector.tensor_tensor(
                        out=ds, in0=w2_sb[:, ct, ns], in1=pbs,
                        op=AluOpType.mult,
                    )
                    if ct < DOTS_ACT:
                        jk2 = junkp.tile([128, 512], BF16, tag="junk2")
                        nc.scalar.activation(
                            out=jk2, in_=ds, func=ACT.Identity,
                            accum_out=dots_parts[:, ct, nn:nn + 1],
                        )
                    else:
                        nc.vector.tensor_reduce(
                            out=dots_parts[:, ct, nn:nn + 1], in_=ds,
                            axis=AX.X, op=AluOpType.add,
                        )
                    # q' = Pi_bc * w  (into the dead w2 slice; all-SBUF bf16
                    # so DVE runs in 2x mode)
                    nc.vector.tensor_tensor(
                        out=w2_sb[:, ct, ns], in0=w_sb[:, ct, ns], in1=pbs,
                        op=AluOpType.mult,
                    )
            q_sb = w2_sb

            # ---------------- negattn; fold into wout ----------------
            dots_c = st.tile([128, CT], F32, tag="dc")
            sinv_c = st.tile([128, CT], F32, tag="sc")
            negattn = st.tile([128, CT], F32, tag="natn")
            nc.vector.tensor_reduce(out=dots_c, in_=dots_parts, axis=AX.X,
                                    op=AluOpType.add)
            nc.vector.tensor_reduce(out=s_sum, in_=s_parts, axis=AX.X,
                                    op=AluOpType.add)
            nc.vector.tensor_scalar_add(out=s_sum, in0=s_sum, scalar1=EPS_PI)
            nc.vector.reciprocal(out=sinv16, in_=s_sum)
            sinvSel = st.tile([H, 8], F32, tag="sinvsel")
            nc.vector.tensor_scalar_mul(out=sinvSel, in0=selH, scalar1=sinv16)
            svp = psS.tile([128, 8], F32, tag="svp")
            nc.tensor.matmul(svp, parityM, sinvSel, start=True, stop=True)
            nc.vector.tensor_copy(out=sinv_c, in_=svp)
            nc.vector.tensor_tensor(out=negattn, in0=dots_c, in1=sinv_c,
                                    op=AluOpType.mult)
            nc.vector.tensor_scalar_add(out=negattn, in0=negattn, scalar1=1.0)
            nc.vector.reciprocal(out=negattn, in_=negattn)
            nc.vector.tensor_scalar_mul(out=negattn, in0=negattn, scalar1=-1.0)
            # woutS = woutT * negattn (per-partition scale), into wq's slot
            woutS = big.tile([128, CT, DIM], BF16, tag="wq")
            for ct in range(CT):
                nc.scalar.activation(
                    out=woutS[:, ct], in_=wout_sb[:, ct], func=ACT.Identity,
                    scale=negattn[:, ct:ct + 1],
                )

            # ---------------- phase 3: y.T = WoutS @ q.T (psum -> DRAM) ----------------
            for nn in range(NCH):
                ns = slice(nn * 512, (nn + 1) * 512)
                for jsub in range(8):
                    yps = psA.tile([128, 512], F32, tag="psA")
                    for ct in range(CT):
                        nc.tensor.matmul(
                            yps,
                            woutS[:, ct, jsub * 128:(jsub + 1) * 128],
                            q_sb[:, ct, ns],
                            start=(ct == 0),
                            stop=(ct == CT - 1),
                        )
                    y_sb = ysbp.tile([128, 512], F32, tag="ysb")
                    nc.vector.tensor_copy(out=y_sb, in_=yps)
                    nc.sync.dma_start(out=y_parts[jsub][:, ns], in_=y_sb)

    nc.finalize()
    return nc


_NC_CACHE = {}


def _get_nc():
    if "nc" not in _NC_CACHE:
        _NC_CACHE["nc"] = build_nc()
    return _NC_CACHE["nc"]


def make_host_inputs(x, Wqkv, temp, Wout, bout):
    """Per-core input maps (host-side sharding, transposes, bf16 casts)."""
    import ml_dtypes

    bf = ml_dtypes.bfloat16
    x = np.asarray(x, dtype=np.float32)
    wqkvT = np.ascontiguousarray(np.asarray(Wqkv, dtype=np.float32).T.astype(bf))
    woutT = np.ascontiguousarray(np.asarray(Wout, dtype=np.float32).T.astype(bf))
    temp = np.ascontiguousarray(np.asarray(temp, dtype=np.float32).reshape(H, 1))
    # maskT[p, ct, h] = 1 iff h == 2*ct + (p >= 64)
    p = np.arange(128)
    maskT = np.zeros((128, CT, H), dtype=np.float32)
    for ct in range(CT):
        maskT[p, ct, 2 * ct + (p >= 64)] = 1.0
    bcastM = np.ascontiguousarray(maskT.transpose(2, 1, 0).astype(bf))
    ones16 = np.ones((H, 1), dtype=bf)
    ones1x16 = np.ones((1, H), dtype=bf)
    parityM = np.zeros((H, 128), dtype=np.float32)
    for h in range(H):
        parityM[h, :] = ((np.arange(128) >= 64) == (h % 2)).astype(np.float32)
    selH = np.zeros((H, 8), dtype=np.float32)
    for h in range(H):
        selH[h, h // 2] = 1.0

    shared = {
        "wqkvT": wqkvT, "woutT": woutT, "temp": temp,
        "maskT": maskT, "bcastM": bcastM,
        "ones16": ones16, "ones1x16": ones1x16,
        "parityM": parityM, "selH": selH,
    }
    xb = np.ascontiguousarray(x.astype(bf))
    maps = []
    for b in range(B):
        m = dict(shared)
        for i in range(NCH):
            m[f"x{i}"] = xb[b, i * 512:(i + 1) * 512]
        maps.append(m)
    return maps


def kernel(x, Wqkv, temp, Wout, bout):
    from concourse.bass_utils import run_bass_kernel_spmd

    nc = _get_nc()
    in_maps = make_host_inputs(x, Wqkv, temp, Wout, bout)
    res = run_bass_kernel_spmd(nc, in_maps, list(range(B)))
    bout_f = np.asarray(bout, dtype=np.float32).reshape(1, DIM)
    y = np.empty((B, N, DIM), dtype=np.float32)
    for b in range(B):
        yt = np.concatenate(
            [res.results[b][f"y{i}"] for i in range(CT)], axis=0
        )
        y[b] = yt.T + bout_f
    return y
